# revision 46
# baseline (speedup 1.0000x reference)
"""Trainium2 Bass kernel for nn_CamadaEquivariante (EGNN message-passing layer).

Strategy (8 NeuronCores, node-sharded):
  Key observation: the reference indexes the EDGE tensors `ex` and `m_ij` by
  the *column node id* (`ex[cols]`, `m_ij[cols]` with cols < N), so only the
  first N=50000 edge-MLP rows are ever consumed.  Per edge j < N we only need
  two scalars downstream:
      s_m[j] = sum_f m_ij[j, f]
      s_x[j] = phi_x[j] * sum_d (x[rows[j]] - x[cols[j]])_d
  and the per-node aggregates are plain gathers + segment sums over the
  (sorted) rows:
      m_i[i]   = sum_{e in seg(i)} s_m[cols[e]]
      media[i] = (1/counts[i]) * sum_{e in seg(i)} s_x[cols[e]]

  Launch A (edge shard j in [c*6250,(c+1)*6250)): dense edge MLP in
    feature-major layout, float32r (tf32) matmuls -> per-edge scalars
    (s_m, s_x) written as a [2, NPAD] table shard.
  Host (reshard between launches): concatenates the 8 scalar shards into a
    [N+1, 2] table (row N = 0 sentinel) and gathers table[cols] per node
    shard -- pure index-driven data movement; TRN2's indirect DMA is
    row-per-partition granular, so a 100K-scalar device gather has no
    efficient primitive.
  Launch B (node shard): segment reduction of the gathered pairs (DVE),
    phi_v / phi_h MLPs (f32r), velocity/coordinate update in node-major
    layout; h_new is emitted feature-major and transposed on the host.
"""

import os
import numpy as np

import concourse.bass as bass
import concourse.bacc as bacc
import concourse.mybir as mybir
import concourse.tile as tile
from concourse.bass import IndirectOffsetOnAxis
from concourse.bass_utils import run_bass_kernel_spmd

NCORES = 8
N = 50000
FIN = 128
HID = 128
FOUT = 128
IJ = 16

SH = N // NCORES          # 6250 nodes/edges per core
PART = 128
SLOT = (SH + PART - 1) // PART   # 49
NPAD = PART * SLOT               # 6272
CHUNKS = [512] * (NPAD // 512) + ([NPAD % 512] if NPAD % 512 else [])

F32 = mybir.dt.float32
F32R = mybir.dt.float32r
I32 = mybir.dt.int32
FT = mybir.ActivationFunctionType
ALU = mybir.AluOpType


def tf32_round(x):
    """Round fp32 -> tf32 (10-bit mantissa) as required for float32r."""
    u = np.ascontiguousarray(x, np.float32).view(np.uint32)
    add = ((u >> 13) & np.uint32(1)) + np.uint32(0x0FFF)
    return ((u + add) & np.uint32(0xFFFFE000)).view(np.float32)

# Results of the traced runs (exec_time_ns etc.), for test.py to inspect.
LAST_RESULTS = []

_BUILD_CACHE = {}


def _bcast_inner(ap, n):
    """View [p, s] AP as [p, s, n] with a stride-0 inner broadcast dim."""
    return bass.AP(ap.tensor, ap.offset, list(ap.ap) + [[0, n]])


def _new_nc():
    return bacc.Bacc(
        "TRN2", target_bir_lowering=False, debug=False, num_devices=NCORES
    )


# --------------------------------------------------------------------------
# Launch A: edge MLP over this core's 6272 (padded) edge rows, feature-major.
# inputs: hrT/hcT [128, NPAD], eaT [16, NPAD], dsqT [3, NPAD], sdrow [1, NPAD]
#         bigw [128, 518], smallw [19, 128]
# output: sout [2, NPAD]  (row 0 = s_m, row 1 = s_x = phi_x * sd)
# bigw columns: 0:128 We1a | 128:256 We1b | 256:384 We2 | 384:512 Wx1 |
#   512 be1 | 513 be2 | 514 bx1 | 515 Wx2 | 516 ones | col 517 row0 = bx2
# smallw rows: 0:16 We1[257:273] | 16:19 tile(We1[256], 3)
# --------------------------------------------------------------------------
def _build_launch_a():
    """Edge MLP, feature-major, chunk-PAIR structured:
    psum tiles span two 512-col chunks (2 banks) so ACT runs 1024-wide.
    inputs: hrT/hcT [128, NPAD] f32r; edgeT [20, NPAD] f32r
      (rows 0:16 = eaT, 16:19 = dsqT, 19 = sd row, fp32-compatible);
    wA [128,512] f32r = We1a|We1b|We2|Wx1; wB [128,2] f32r = Wx2|ones;
    bias [128,4] f32 = be1|be2|bx1|bx2@[0,3]; smallw [19,128] f32r.
    output: sout [2, NPAD] (s_m, s_x)."""
    if "A" in _BUILD_CACHE:
        return _BUILD_CACHE["A"]
    nc = _new_nc()
    hrT_d = nc.dram_tensor("hrT", [PART, NPAD], F32R, kind="ExternalInput")
    hcT_d = nc.dram_tensor("hcT", [PART, NPAD], F32R, kind="ExternalInput")
    edgeT_d = nc.dram_tensor("edgeT", [IJ + 4, NPAD], F32R, kind="ExternalInput")
    wA_d = nc.dram_tensor("wA", [PART, 512], F32R, kind="ExternalInput")
    wB_d = nc.dram_tensor("wB", [PART, 2], F32R, kind="ExternalInput")
    bias_d = nc.dram_tensor("bias", [PART, 4], F32, kind="ExternalInput")
    smallw_d = nc.dram_tensor("smallw", [IJ + 3, PART], F32R, kind="ExternalInput")
    sout_d = nc.dram_tensor("sout", [2, NPAD], F32, kind="ExternalOutput")

    # chunk pairs: [(off, cs0, cs1)], cs1 may be 0 for the tail
    pairs = []
    off = 0
    i = 0
    while i < len(CHUNKS):
        cs0 = CHUNKS[i]
        cs1 = CHUNKS[i + 1] if i + 1 < len(CHUNKS) else 0
        pairs.append((off, cs0, cs1))
        off += cs0 + cs1
        i += 2

    with tile.TileContext(nc) as tc:
        with (
            tc.tile_pool(name="cons", bufs=1) as cons,
            tc.tile_pool(name="acts", bufs=1) as acts,
            tc.tile_pool(name="work", bufs=2) as work,
            tc.tile_pool(name="rows", bufs=2) as rowsp,
            tc.tile_pool(name="ps", bufs=2, space="PSUM") as ps,
            tc.tile_pool(name="ps2", bufs=2, space="PSUM") as ps2,
            tc.tile_pool(name="ps3", bufs=2, space="PSUM") as ps3,
        ):
            wA = cons.tile([PART, 512], F32R, tag="wA")
            nc.sync.dma_start(wA[:, :], wA_d[:, :])
            wB = cons.tile([PART, 2], F32R, tag="wB")
            nc.sync.dma_start(wB[:, :], wB_d[:, :])
            bias = cons.tile([PART, 4], F32, tag="bias")
            nc.sync.dma_start(bias[:, :], bias_d[:, :])
            sw1 = cons.tile([IJ, PART], F32R, tag="sw1")
            nc.sync.dma_start(sw1[:, :], smallw_d[0:IJ, :])
            sw2 = cons.tile([3, PART], F32R, tag="sw2")
            nc.sync.dma_start(sw2[:, :], smallw_d[IJ:IJ + 3, :])
            # small tensors first so L1's ea/dsq matmuls never stall
            ea = acts.tile([IJ, NPAD], F32R, tag="ea")
            nc.sync.dma_start(ea[:, :], edgeT_d[0:IJ, :])
            dsq = acts.tile([3, NPAD], F32R, tag="dsq")
            nc.sync.dma_start(dsq[:, :], edgeT_d[IJ:IJ + 3, :])
            sdrt = acts.tile([1, NPAD], F32R, tag="sdrt")
            nc.sync.dma_start(sdrt[:, :], edgeT_d[IJ + 3:IJ + 4, :])
            sdr = sdrt[0:1, :].bitcast(F32)
            # h halo tensors split in halves: first pairs start early
            HALF = 7 * 512
            hr = acts.tile([PART, NPAD], F32R, tag="hr")
            hc = acts.tile([PART, NPAD], F32R, tag="hc")
            nc.sync.dma_start(hr[:, 0:HALF], hrT_d[:, 0:HALF])
            nc.sync.dma_start(hc[:, 0:HALF], hcT_d[:, 0:HALF])
            nc.sync.dma_start(hr[:, HALF:], hrT_d[:, HALF:])
            nc.sync.dma_start(hc[:, HALF:], hcT_d[:, HALF:])

            a1f = acts.tile([PART, NPAD], F32R, tag="a1f")

            # phase 1: all L1 matmuls back-to-back (dense PE stream -> HAM
            # ramps), wide tanh ACTs trail into the materialized a1f
            for off, cs0, cs1 in pairs:
                pw = cs0 + cs1
                halves = [(off, 0, cs0)] + ([(off + cs0, cs0, cs1)] if cs1 else [])
                p1 = ps.tile([PART, 1024], F32, tag="p1")
                for ho, po, hcs in halves:
                    r = slice(ho, ho + hcs)
                    o = slice(po, po + hcs)
                    nc.tensor.matmul(p1[:, o], wA[:, 0:128], hr[:, r],
                                     start=True, stop=False)
                    nc.tensor.matmul(p1[:, o], wA[:, 128:256], hc[:, r],
                                     start=False, stop=False)
                    nc.tensor.matmul(p1[:, o], sw1[:, :], ea[:, r],
                                     start=False, stop=False)
                    nc.tensor.matmul(p1[:, o], sw2[:, :], dsq[:, r],
                                     start=False, stop=True)
                nc.scalar.activation(a1f[:, off:off + pw], p1[:, :pw], FT.Tanh,
                                     bias=bias[:, 0:1])

            # phase 2: dependent L2 -> X1 -> row chain per 512-chunk
            off = 0
            for cs in CHUNKS:
                sl = slice(off, off + cs)
                p2 = ps2.tile([PART, 512], F32, tag="p2")
                nc.tensor.matmul(p2[:, :cs], wA[:, 256:384], a1f[:, sl],
                                 start=True, stop=True)
                m0 = work.tile([PART, 512], F32R, tag="m0")
                nc.scalar.activation(m0[:, :cs], p2[:, :cs], FT.Tanh,
                                     bias=bias[:, 1:2])
                p3 = ps3.tile([PART, 512], F32, tag="p3")
                nc.tensor.matmul(p3[:, :cs], wA[:, 384:512], m0[:, :cs],
                                 start=True, stop=True)
                ax = work.tile([PART, 512], F32R, tag="ax")
                nc.scalar.activation(ax[:, :cs], p3[:, :cs], FT.Tanh,
                                     bias=bias[:, 2:3])
                nc.tensor.matmul(p3[0:1, :cs], wB[:, 0:1], ax[:, :cs],
                                 start=True, stop=True)
                nc.tensor.matmul(p2[0:1, :cs], wB[:, 1:2], m0[:, :cs],
                                 start=True, stop=True)
                pxc = rowsp.tile([1, 512], F32, tag="pxc")
                nc.scalar.activation(pxc[:, :cs], p3[0:1, :cs], FT.Tanh,
                                     bias=bias[0:1, 3:4])
                nc.vector.tensor_tensor(pxc[:, :cs], pxc[:, :cs],
                                        sdr[:, sl], op=ALU.mult)
                smc = rowsp.tile([1, 512], F32, tag="smc")
                nc.vector.tensor_copy(smc[:, :cs], p2[0:1, :cs])
                nc.sync.dma_start(sout_d[0:1, sl], smc[:, :cs])
                nc.sync.dma_start(sout_d[1:2, sl], pxc[:, :cs])
                off += cs

    nc.compile()
    _BUILD_CACHE["A"] = nc
    return nc


def _build_launch_b(K):
    key = ("B", K)
    if key in _BUILD_CACHE:
        return _BUILD_CACHE[key]
    nc = _new_nc()
    NIDX = SLOT * K
    hT_d = nc.dram_tensor("hT", [PART, NPAD], F32R, kind="ExternalInput")
    xnl_d = nc.dram_tensor("xnl", [PART, SLOT * 3], F32, kind="ExternalInput")
    vnl_d = nc.dram_tensor("vnl", [PART, SLOT * 3], F32, kind="ExternalInput")
    recip_d = nc.dram_tensor("recip", [PART, SLOT], F32, kind="ExternalInput")
    gt_d = nc.dram_tensor("gt", [PART, NIDX * 2], F32, kind="ExternalInput")
    wA2_d = nc.dram_tensor("wA2", [PART, 384], F32R, kind="ExternalInput")
    wB2_d = nc.dram_tensor("wB2", [PART, 1], F32R, kind="ExternalInput")
    bias2_d = nc.dram_tensor("bias2", [PART, 4], F32, kind="ExternalInput")
    wrow_d = nc.dram_tensor("wrow", [1, 128], F32R, kind="ExternalInput")
    hnew_d = nc.dram_tensor("hnew", [PART, NPAD], F32, kind="ExternalOutput")
    xnew_d = nc.dram_tensor("xnew", [NPAD, 3], F32, kind="ExternalOutput")
    vnew_d = nc.dram_tensor("vnew", [NPAD, 3], F32, kind="ExternalOutput")
    dbg = bool(int(os.environ.get("KERNEL_DEBUG", "0")))
    if dbg:
        mrow_d = nc.dram_tensor("mrowdbg", [1, NPAD], F32, kind="ExternalOutput")
        mi_d = nc.dram_tensor("midbg", [PART, SLOT], F32, kind="ExternalOutput")

    pairs = []
    off = 0
    i = 0
    while i < len(CHUNKS):
        cs0 = CHUNKS[i]
        cs1 = CHUNKS[i + 1] if i + 1 < len(CHUNKS) else 0
        pairs.append((off, cs0, cs1))
        off += cs0 + cs1
        i += 2

    with tile.TileContext(nc) as tc:
        with (
            tc.tile_pool(name="cons", bufs=1) as cons,
            tc.tile_pool(name="acts", bufs=1) as acts,
            tc.tile_pool(name="work", bufs=2) as work,
            tc.tile_pool(name="ps", bufs=2, space="PSUM") as ps,
            tc.tile_pool(name="ps2", bufs=2, space="PSUM") as ps2,
        ):
            wA2 = cons.tile([PART, 384], F32R, tag="wA2")
            nc.sync.dma_start(wA2[:, :], wA2_d[:, :])
            wB2 = cons.tile([PART, 1], F32R, tag="wB2")
            nc.sync.dma_start(wB2[:, :], wB2_d[:, :])
            bias2 = cons.tile([PART, 4], F32, tag="bias2")
            nc.sync.dma_start(bias2[:, :], bias2_d[:, :])
            wrow = cons.tile([1, 128], F32R, tag="wrow")
            nc.sync.dma_start(wrow[:, :], wrow_d[:, :])
            # gather pairs + small tensors first: the reduce chain that
            # gates phi_h must not queue behind the 3.2MB hT load
            gt = acts.tile([PART, NIDX * 2], F32, tag="gt")
            nc.sync.dma_start(gt[:, :], gt_d[:, :])
            recip = acts.tile([PART, SLOT], F32, tag="recip")
            nc.sync.dma_start(recip[:, :], recip_d[:, :])
            xnl = acts.tile([PART, SLOT * 3], F32, tag="xnl")
            nc.sync.dma_start(xnl[:, :], xnl_d[:, :])
            vnl = acts.tile([PART, SLOT * 3], F32, tag="vnl")
            nc.sync.dma_start(vnl[:, :], vnl_d[:, :])
            hT = acts.tile([PART, NPAD], F32R, tag="hT")
            for poff, pcs0, pcs1 in pairs:
                psl = slice(poff, poff + pcs0 + pcs1)
                nc.sync.dma_start(hT[:, psl], hT_d[:, psl])

            pvrow = acts.tile([1, NPAD], F32, tag="pvrow")
            mrow = acts.tile([1, NPAD], F32R, tag="mrow")

            # ---- segment reduction of gathered pairs (DVE, early) ----
            gt_r = gt[:, :].rearrange("p (s k t) -> p s k t", s=SLOT, k=K, t=2)
            m_i = acts.tile([PART, SLOT], F32, tag="m_i")
            nc.vector.tensor_reduce(m_i[:, :], gt_r[:, :, :, 0],
                                    axis=mybir.AxisListType.X, op=ALU.add)
            mr = acts.tile([PART, SLOT], F32, tag="mr")
            nc.vector.tensor_reduce(mr[:, :], gt_r[:, :, :, 1],
                                    axis=mybir.AxisListType.X, op=ALU.add)
            media = acts.tile([PART, SLOT], F32, tag="media")
            nc.vector.tensor_tensor(media[:, :], mr[:, :], recip[:, :],
                                    op=ALU.mult)
            m_ir = acts.tile([PART, SLOT], F32R, tag="m_ir")
            nc.vector.tensor_copy(m_ir[:, :], m_i[:, :])
            nc.sync.dma_start(mrow[:, :], m_ir[:, :])

            # ---- phi_v over all pairs (dense PE stream) ----
            for off, cs0, cs1 in pairs:
                pw = cs0 + cs1
                halves = [(off, 0, cs0)] + ([(off + cs0, cs0, cs1)] if cs1 else [])
                pA = ps.tile([PART, 1024], F32, tag="mm")
                for ho, po, hcs in halves:
                    r = slice(ho, ho + hcs)
                    o = slice(po, po + hcs)
                    nc.tensor.matmul(pA[:, o], wA2[:, 0:128], hT[:, r],
                                     start=True, stop=True)
                av = work.tile([PART, 1024], F32R, tag="av")
                nc.scalar.activation(av[:, :pw], pA[:, :pw], FT.Tanh,
                                     bias=bias2[:, 0:1])
                for ho, po, hcs in halves:
                    o = slice(po, po + hcs)
                    nc.tensor.matmul(pA[0:1, o], wB2[:, 0:1], av[:, o],
                                     start=True, stop=True)
                nc.vector.tensor_scalar_add(pvrow[:, off:off + pw],
                                            pA[0:1, :pw], bias2[0:1, 3:4])


            # ---- node-major reshuffle + velocity / position update ----
            pvnl = acts.tile([PART, SLOT], F32, tag="pvnl")
            nc.sync.dma_start(pvnl[:, :], pvrow[:, :])
            vn = acts.tile([PART, SLOT * 3], F32, tag="vn")
            xn = acts.tile([PART, SLOT * 3], F32, tag="xn")
            vn_r = vn[:, :].rearrange("p (s t) -> p s t", s=SLOT, t=3)
            xn_r = xn[:, :].rearrange("p (s t) -> p s t", s=SLOT, t=3)
            v_r = vnl[:, :].rearrange("p (s t) -> p s t", s=SLOT, t=3)
            x_r = xnl[:, :].rearrange("p (s t) -> p s t", s=SLOT, t=3)
            nc.vector.tensor_tensor(vn_r, v_r, _bcast_inner(pvnl[:, :], 3),
                                    op=ALU.mult)
            nc.vector.tensor_tensor(vn_r, vn_r, _bcast_inner(media[:, :], 3),
                                    op=ALU.add)
            nc.vector.tensor_tensor(xn_r, x_r, vn_r, op=ALU.add)
            nc.sync.dma_start(
                vnew_d.ap().rearrange("(p s) t -> p (s t)", p=PART), vn[:, :]
            )
            nc.sync.dma_start(
                xnew_d.ap().rearrange("(p s) t -> p (s t)", p=PART), xn[:, :]
            )

            # ---- phi_h over chunk pairs ----
            for off, cs0, cs1 in pairs:
                pw = cs0 + cs1
                halves = [(off, 0, cs0)] + ([(off + cs0, cs0, cs1)] if cs1 else [])
                pC = ps.tile([PART, 1024], F32, tag="mm")
                for ho, po, hcs in halves:
                    r = slice(ho, ho + hcs)
                    o = slice(po, po + hcs)
                    nc.tensor.matmul(pC[:, o], wA2[:, 128:256], hT[:, r],
                                     start=True, stop=False)
                    nc.tensor.matmul(pC[:, o], wrow[0:1, :], mrow[:, r],
                                     start=False, stop=True)
                ah = work.tile([PART, 1024], F32R, tag="ah")
                nc.scalar.activation(ah[:, :pw], pC[:, :pw], FT.Tanh,
                                     bias=bias2[:, 1:2])
                pD = ps2.tile([PART, 1024], F32, tag="mmD")
                for ho, po, hcs in halves:
                    o = slice(po, po + hcs)
                    nc.tensor.matmul(pD[:, o], wA2[:, 256:384], ah[:, o],
                                     start=True, stop=True)
                hnc = work.tile([PART, 1024], F32, tag="hnc")
                nc.vector.tensor_scalar_add(hnc[:, :pw], pD[:, :pw],
                                            bias2[:, 2:3])
                nc.sync.dma_start(hnew_d[:, off:off + pw], hnc[:, :pw])

            if dbg:
                nc.sync.dma_start(mrow_d.ap(), mrow[:, :])
                nc.sync.dma_start(mi_d.ap(), m_i[:, :])

    nc.compile()
    _BUILD_CACHE[key] = nc
    return nc


def _trace_flag():
    return bool(int(os.environ.get("KERNEL_TRACE", "0")))


def kernel(h, x, velocidade, atributos_arestas, rows, cols,
           We1, be1, We2, be2, Wx1, bx1, Wx2, bx2,
           Wh1, bh1, Wh2, bh2, Wv1, bv1, Wv2, bv2):
    LAST_RESULTS.clear()
    h = np.ascontiguousarray(np.asarray(h, dtype=np.float32))
    x = np.ascontiguousarray(np.asarray(x, dtype=np.float32))
    vel = np.ascontiguousarray(np.asarray(velocidade, dtype=np.float32))
    ea = np.asarray(atributos_arestas, dtype=np.float32)
    rows = np.asarray(rows).astype(np.int64)
    cols = np.asarray(cols).astype(np.int64)
    Ws = {k: np.asarray(v, dtype=np.float32) for k, v in dict(
        We1=We1, be1=be1, We2=We2, be2=be2, Wx1=Wx1, bx1=bx1, Wx2=Wx2,
        bx2=bx2, Wh1=Wh1, bh1=bh1, Wh2=Wh2, bh2=bh2, Wv1=Wv1, bv1=bv1,
        Wv2=Wv2, bv2=bv2).items()}
    E = rows.shape[0]
    assert h.shape == (N, FIN) and E >= N

    trace = _trace_flag()
    cores = list(range(NCORES))

    # ---------------- host-side sharding / halo gather ----------------
    r0 = rows[:N]
    c0 = cols[:N]
    diff0 = x[r0] - x[c0]                         # [N, 3]
    dsq0 = diff0 * diff0
    sd0 = diff0.sum(axis=1).astype(np.float32)    # [N]

    # Launch A weight stacks (shared by all cores); matmul weights are
    # pre-rounded to tf32 for the float32r PE fast path
    wA = np.zeros((PART, 512), np.float32)
    wA[:, 0:128] = Ws["We1"][0:128]
    wA[:, 128:256] = Ws["We1"][128:256]
    wA[:, 256:384] = Ws["We2"]
    wA[:, 384:512] = Ws["Wx1"]
    wA = tf32_round(wA)
    wB = np.zeros((PART, 2), np.float32)
    wB[:, 0] = Ws["Wx2"][:, 0]
    wB[:, 1] = 1.0
    wB = tf32_round(wB)
    bias = np.zeros((PART, 4), np.float32)
    bias[:, 0] = Ws["be1"]
    bias[:, 1] = Ws["be2"]
    bias[:, 2] = Ws["bx1"]
    bias[0, 3] = Ws["bx2"][0]
    smallw = np.zeros((IJ + 3, PART), np.float32)
    smallw[0:IJ] = Ws["We1"][257:273]
    smallw[IJ:IJ + 3] = np.tile(Ws["We1"][256:257], (3, 1))
    smallw = tf32_round(smallw)

    def padT(a2d):  # [rows<=SH, d] -> [d, NPAD] transposed + padded
        out = np.zeros((a2d.shape[1], NPAD), np.float32)
        out[:, :a2d.shape[0]] = a2d.T
        return out

    in_maps_a = []
    for c in cores:
        sl = slice(c * SH, (c + 1) * SH)
        edgeT = np.zeros((IJ + 4, NPAD), np.float32)
        edgeT[0:IJ] = tf32_round(padT(ea[:N][sl]))
        edgeT[IJ:IJ + 3] = tf32_round(padT(dsq0[sl]))
        edgeT[IJ + 3] = padT(sd0[sl, None])[0]
        in_maps_a.append({
            "hrT": tf32_round(padT(h[r0[sl]])),
            "hcT": tf32_round(padT(h[c0[sl]])),
            "edgeT": edgeT,
            "wA": wA,
            "wB": wB,
            "bias": bias,
            "smallw": smallw,
        })

    nc_a = _build_launch_a()
    res_a = run_bass_kernel_spmd(nc_a, in_maps_a, core_ids=cores, trace=trace)
    LAST_RESULTS.append(res_a)

    # ---------------- assemble gather table (host reshard) ----------------
    table = np.zeros((N + 1, 2), np.float32)
    for c in cores:
        so = res_a.results[c]["sout"]            # [2, NPAD]
        table[c * SH:(c + 1) * SH, 0] = so[0, :SH]
        table[c * SH:(c + 1) * SH, 1] = so[1, :SH]

    # ---------------- launch B host prep ----------------
    counts = np.bincount(rows, minlength=N).astype(np.int64)
    uniform = E == N * 16 and (counts == 16).all()
    if uniform:
        K = 16
        idx_full = cols.reshape(N, 16)
        pad_mask = None
    else:
        K = max(int(counts.max()), 1)
        K += (-K) % 2
        idx_full = np.full((N, K), N, np.int64)
        offs = np.zeros(N + 1, np.int64)
        np.cumsum(counts, out=offs[1:])
        ar = np.arange(E) - offs[rows]
        idx_full[rows, ar] = cols
    recip_full = np.zeros(N, np.float32)
    nz = counts > 0
    recip_full[nz] = 1.0 / counts[nz]

    # Launch B weights
    wA2 = np.zeros((PART, 384), np.float32)
    wA2[:, 0:128] = Ws["Wv1"]
    wA2[:, 128:256] = Ws["Wh1"][0:128]
    wA2[:, 256:384] = Ws["Wh2"]
    wA2 = tf32_round(wA2)
    wB2 = tf32_round(Ws["Wv2"].reshape(PART, 1))
    bias2 = np.zeros((PART, 4), np.float32)
    bias2[:, 0] = Ws["bv1"]
    bias2[:, 1] = Ws["bh1"]
    bias2[:, 2] = Ws["bh2"]
    bias2[0, 3] = Ws["bv2"][0]
    wrow = tf32_round(Ws["Wh1"][128].reshape(1, 128))

    def node_layout(a, d):  # [<=SH, d] -> [128, SLOT*d]
        out = np.zeros((NPAD, d), np.float32)
        out[:a.shape[0]] = a.reshape(a.shape[0], d)
        return out.reshape(PART, SLOT * d)

    in_maps_b = []
    for c in cores:
        sl = slice(c * SH, (c + 1) * SH)
        idx_sh = np.full((NPAD, K), N, np.int64)
        idx_sh[:SH] = idx_full[sl]
        # host-side value gather (index-driven reshard of launch-A output)
        gt_host = table[idx_sh.reshape(-1)].reshape(PART, SLOT * K * 2)
        in_maps_b.append({
            "hT": tf32_round(padT(h[sl])),
            "xnl": node_layout(x[sl], 3),
            "vnl": node_layout(vel[sl], 3),
            "recip": node_layout(recip_full[sl, None], 1),
            "gt": gt_host,
            "wA2": wA2,
            "wB2": wB2,
            "bias2": bias2,
            "wrow": wrow,
        })

    nc_b = _build_launch_b(K)
    res_b = run_bass_kernel_spmd(nc_b, in_maps_b, core_ids=cores, trace=trace)
    LAST_RESULTS.append(res_b)

    # ---------------- unshard ----------------
    h_new = np.empty((N, FOUT), np.float32)
    x_new = np.empty((N, 3), np.float32)
    v_new = np.empty((N, 3), np.float32)
    for c in cores:
        r = res_b.results[c]
        sl = slice(c * SH, (c + 1) * SH)
        h_new[sl] = r["hnew"][:, :SH].T
        x_new[sl] = r["xnew"][:SH]
        v_new[sl] = r["vnew"][:SH]
    return h_new, x_new, v_new


# revision 47
# speedup vs baseline: 1.0462x; 1.0462x over previous
"""Trainium2 Bass kernel for nn_CamadaEquivariante (EGNN message-passing layer).

Strategy (8 NeuronCores, node-sharded):
  Key observation: the reference indexes the EDGE tensors `ex` and `m_ij` by
  the *column node id* (`ex[cols]`, `m_ij[cols]` with cols < N), so only the
  first N=50000 edge-MLP rows are ever consumed.  Per edge j < N we only need
  two scalars downstream:
      s_m[j] = sum_f m_ij[j, f]
      s_x[j] = phi_x[j] * sum_d (x[rows[j]] - x[cols[j]])_d
  and the per-node aggregates are plain gathers + segment sums over the
  (sorted) rows:
      m_i[i]   = sum_{e in seg(i)} s_m[cols[e]]
      media[i] = (1/counts[i]) * sum_{e in seg(i)} s_x[cols[e]]

  Launch A (edge shard j in [c*6250,(c+1)*6250)): dense edge MLP in
    feature-major layout, float32r (tf32) matmuls -> per-edge scalars
    (s_m, s_x) written as a [2, NPAD] table shard.
  Host (reshard between launches): concatenates the 8 scalar shards into a
    [N+1, 2] table (row N = 0 sentinel) and gathers table[cols] per node
    shard -- pure index-driven data movement; TRN2's indirect DMA is
    row-per-partition granular, so a 100K-scalar device gather has no
    efficient primitive.
  Launch B (node shard): segment reduction of the gathered pairs (DVE),
    phi_v / phi_h MLPs (f32r), velocity/coordinate update in node-major
    layout; h_new is emitted feature-major and transposed on the host.
"""

import os
import numpy as np

import concourse.bass as bass
import concourse.bacc as bacc
import concourse.mybir as mybir
import concourse.tile as tile
from concourse.bass import IndirectOffsetOnAxis
from concourse.bass_utils import run_bass_kernel_spmd

NCORES = 8
N = 50000
FIN = 128
HID = 128
FOUT = 128
IJ = 16

SH = N // NCORES          # 6250 nodes/edges per core
PART = 128
SLOT = (SH + PART - 1) // PART   # 49
NPAD = PART * SLOT               # 6272
CHUNKS = [512] * (NPAD // 512) + ([NPAD % 512] if NPAD % 512 else [])

F32 = mybir.dt.float32
F32R = mybir.dt.float32r
I32 = mybir.dt.int32
FT = mybir.ActivationFunctionType
ALU = mybir.AluOpType


def tf32_round(x):
    """Round fp32 -> tf32 (10-bit mantissa) as required for float32r."""
    u = np.ascontiguousarray(x, np.float32).view(np.uint32)
    add = ((u >> 13) & np.uint32(1)) + np.uint32(0x0FFF)
    return ((u + add) & np.uint32(0xFFFFE000)).view(np.float32)

# Results of the traced runs (exec_time_ns etc.), for test.py to inspect.
LAST_RESULTS = []

_BUILD_CACHE = {}


def _bcast_inner(ap, n):
    """View [p, s] AP as [p, s, n] with a stride-0 inner broadcast dim."""
    return bass.AP(ap.tensor, ap.offset, list(ap.ap) + [[0, n]])


def _new_nc():
    return bacc.Bacc(
        "TRN2", target_bir_lowering=False, debug=False, num_devices=NCORES
    )


# --------------------------------------------------------------------------
# Launch A: edge MLP over this core's 6272 (padded) edge rows, feature-major.
# inputs: hrT/hcT [128, NPAD], eaT [16, NPAD], dsqT [3, NPAD], sdrow [1, NPAD]
#         bigw [128, 518], smallw [19, 128]
# output: sout [2, NPAD]  (row 0 = s_m, row 1 = s_x = phi_x * sd)
# bigw columns: 0:128 We1a | 128:256 We1b | 256:384 We2 | 384:512 Wx1 |
#   512 be1 | 513 be2 | 514 bx1 | 515 Wx2 | 516 ones | col 517 row0 = bx2
# smallw rows: 0:16 We1[257:273] | 16:19 tile(We1[256], 3)
# --------------------------------------------------------------------------
def _build_launch_a():
    """Edge MLP, feature-major, chunk-PAIR structured:
    psum tiles span two 512-col chunks (2 banks) so ACT runs 1024-wide.
    inputs: hrT/hcT [128, NPAD] f32r; edgeT [20, NPAD] f32r
      (rows 0:16 = eaT, 16:19 = dsqT, 19 = sd row, fp32-compatible);
    wA [128,512] f32r = We1a|We1b|We2|Wx1; wB [128,2] f32r = Wx2|ones;
    bias [128,4] f32 = be1|be2|bx1|bx2@[0,3]; smallw [19,128] f32r.
    output: sout [2, NPAD] (s_m, s_x)."""
    if "A" in _BUILD_CACHE:
        return _BUILD_CACHE["A"]
    nc = _new_nc()
    hrT_d = nc.dram_tensor("hrT", [PART, NPAD], F32R, kind="ExternalInput")
    hcT_d = nc.dram_tensor("hcT", [PART, NPAD], F32R, kind="ExternalInput")
    edgeT_d = nc.dram_tensor("edgeT", [IJ + 4, NPAD], F32R, kind="ExternalInput")
    wA_d = nc.dram_tensor("wA", [PART, 512], F32R, kind="ExternalInput")
    wB_d = nc.dram_tensor("wB", [PART, 2], F32R, kind="ExternalInput")
    bias_d = nc.dram_tensor("bias", [PART, 4], F32, kind="ExternalInput")
    smallw_d = nc.dram_tensor("smallw", [IJ + 3, PART], F32R, kind="ExternalInput")
    sout_d = nc.dram_tensor("sout", [2, NPAD], F32, kind="ExternalOutput")

    # chunk pairs: [(off, cs0, cs1)], cs1 may be 0 for the tail
    pairs = []
    off = 0
    i = 0
    while i < len(CHUNKS):
        cs0 = CHUNKS[i]
        cs1 = CHUNKS[i + 1] if i + 1 < len(CHUNKS) else 0
        pairs.append((off, cs0, cs1))
        off += cs0 + cs1
        i += 2

    with tile.TileContext(nc) as tc:
        with (
            tc.tile_pool(name="cons", bufs=1) as cons,
            tc.tile_pool(name="acts", bufs=1) as acts,
            tc.tile_pool(name="work", bufs=3) as work,
            tc.tile_pool(name="rows", bufs=4) as rowsp,
            tc.tile_pool(name="ps", bufs=2, space="PSUM") as ps,
            tc.tile_pool(name="ps2", bufs=2, space="PSUM") as ps2,
            tc.tile_pool(name="ps3", bufs=2, space="PSUM") as ps3,
        ):
            wA = cons.tile([PART, 512], F32R, tag="wA")
            nc.sync.dma_start(wA[:, :], wA_d[:, :])
            wB = cons.tile([PART, 2], F32R, tag="wB")
            nc.sync.dma_start(wB[:, :], wB_d[:, :])
            bias = cons.tile([PART, 4], F32, tag="bias")
            nc.sync.dma_start(bias[:, :], bias_d[:, :])
            sw1 = cons.tile([IJ, PART], F32R, tag="sw1")
            nc.sync.dma_start(sw1[:, :], smallw_d[0:IJ, :])
            sw2 = cons.tile([3, PART], F32R, tag="sw2")
            nc.sync.dma_start(sw2[:, :], smallw_d[IJ:IJ + 3, :])
            # small tensors first so L1's ea/dsq matmuls never stall
            ea = acts.tile([IJ, NPAD], F32R, tag="ea")
            nc.sync.dma_start(ea[:, :], edgeT_d[0:IJ, :])
            dsq = acts.tile([3, NPAD], F32R, tag="dsq")
            nc.sync.dma_start(dsq[:, :], edgeT_d[IJ:IJ + 3, :])
            sdrt = acts.tile([1, NPAD], F32R, tag="sdrt")
            nc.sync.dma_start(sdrt[:, :], edgeT_d[IJ + 3:IJ + 4, :])
            sdr = sdrt[0:1, :].bitcast(F32)
            # h halo tensors split in halves: first pairs start early
            HALF = 7 * 512
            hr = acts.tile([PART, NPAD], F32R, tag="hr")
            hc = acts.tile([PART, NPAD], F32R, tag="hc")
            nc.sync.dma_start(hr[:, 0:HALF], hrT_d[:, 0:HALF])
            nc.sync.dma_start(hc[:, 0:HALF], hcT_d[:, 0:HALF])
            nc.sync.dma_start(hr[:, HALF:], hrT_d[:, HALF:])
            nc.sync.dma_start(hc[:, HALF:], hcT_d[:, HALF:])

            a1f = acts.tile([PART, NPAD], F32R, tag="a1f")

            # phase 1: all L1 matmuls back-to-back (dense PE stream -> HAM
            # ramps), wide tanh ACTs trail into the materialized a1f
            for off, cs0, cs1 in pairs:
                pw = cs0 + cs1
                halves = [(off, 0, cs0)] + ([(off + cs0, cs0, cs1)] if cs1 else [])
                p1 = ps.tile([PART, 1024], F32, tag="p1")
                for ho, po, hcs in halves:
                    r = slice(ho, ho + hcs)
                    o = slice(po, po + hcs)
                    nc.tensor.matmul(p1[:, o], wA[:, 0:128], hr[:, r],
                                     start=True, stop=False)
                    nc.tensor.matmul(p1[:, o], wA[:, 128:256], hc[:, r],
                                     start=False, stop=False)
                    nc.tensor.matmul(p1[:, o], sw1[:, :], ea[:, r],
                                     start=False, stop=False)
                    nc.tensor.matmul(p1[:, o], sw2[:, :], dsq[:, r],
                                     start=False, stop=True)
                nc.scalar.activation(a1f[:, off:off + pw], p1[:, :pw], FT.Tanh,
                                     bias=bias[:, 0:1])

            # phase 2: dependent L2 -> X1 -> row chain per 512-chunk
            off = 0
            for cs in CHUNKS:
                sl = slice(off, off + cs)
                p2 = ps2.tile([PART, 512], F32, tag="p2")
                nc.tensor.matmul(p2[:, :cs], wA[:, 256:384], a1f[:, sl],
                                 start=True, stop=True)
                m0 = work.tile([PART, 512], F32R, tag="m0")
                nc.scalar.activation(m0[:, :cs], p2[:, :cs], FT.Tanh,
                                     bias=bias[:, 1:2])
                p3 = ps3.tile([PART, 512], F32, tag="p3")
                nc.tensor.matmul(p3[:, :cs], wA[:, 384:512], m0[:, :cs],
                                 start=True, stop=True)
                ax = work.tile([PART, 512], F32R, tag="ax")
                nc.scalar.activation(ax[:, :cs], p3[:, :cs], FT.Tanh,
                                     bias=bias[:, 2:3])
                nc.tensor.matmul(p3[0:1, :cs], wB[:, 0:1], ax[:, :cs],
                                 start=True, stop=True)
                nc.tensor.matmul(p2[0:1, :cs], wB[:, 1:2], m0[:, :cs],
                                 start=True, stop=True)
                pxc = rowsp.tile([1, 512], F32, tag="pxc")
                nc.scalar.activation(pxc[:, :cs], p3[0:1, :cs], FT.Tanh,
                                     bias=bias[0:1, 3:4])
                nc.vector.tensor_tensor(pxc[:, :cs], pxc[:, :cs],
                                        sdr[:, sl], op=ALU.mult)
                smc = rowsp.tile([1, 512], F32, tag="smc")
                nc.vector.tensor_copy(smc[:, :cs], p2[0:1, :cs])
                nc.sync.dma_start(sout_d[0:1, sl], smc[:, :cs])
                nc.sync.dma_start(sout_d[1:2, sl], pxc[:, :cs])
                off += cs

    nc.compile()
    _BUILD_CACHE["A"] = nc
    return nc


def _build_launch_b(K):
    key = ("B", K)
    if key in _BUILD_CACHE:
        return _BUILD_CACHE[key]
    nc = _new_nc()
    NIDX = SLOT * K
    hT_d = nc.dram_tensor("hT", [PART, NPAD], F32R, kind="ExternalInput")
    xnl_d = nc.dram_tensor("xnl", [PART, SLOT * 3], F32, kind="ExternalInput")
    vnl_d = nc.dram_tensor("vnl", [PART, SLOT * 3], F32, kind="ExternalInput")
    recip_d = nc.dram_tensor("recip", [PART, SLOT], F32, kind="ExternalInput")
    gt_d = nc.dram_tensor("gt", [PART, NIDX * 2], F32, kind="ExternalInput")
    wA2_d = nc.dram_tensor("wA2", [PART, 384], F32R, kind="ExternalInput")
    wB2_d = nc.dram_tensor("wB2", [PART, 1], F32R, kind="ExternalInput")
    bias2_d = nc.dram_tensor("bias2", [PART, 4], F32, kind="ExternalInput")
    wrow_d = nc.dram_tensor("wrow", [1, 128], F32R, kind="ExternalInput")
    hnew_d = nc.dram_tensor("hnew", [PART, NPAD], F32, kind="ExternalOutput")
    xnew_d = nc.dram_tensor("xnew", [NPAD, 3], F32, kind="ExternalOutput")
    vnew_d = nc.dram_tensor("vnew", [NPAD, 3], F32, kind="ExternalOutput")
    dbg = bool(int(os.environ.get("KERNEL_DEBUG", "0")))
    if dbg:
        mrow_d = nc.dram_tensor("mrowdbg", [1, NPAD], F32, kind="ExternalOutput")
        mi_d = nc.dram_tensor("midbg", [PART, SLOT], F32, kind="ExternalOutput")

    pairs = []
    off = 0
    i = 0
    while i < len(CHUNKS):
        cs0 = CHUNKS[i]
        cs1 = CHUNKS[i + 1] if i + 1 < len(CHUNKS) else 0
        pairs.append((off, cs0, cs1))
        off += cs0 + cs1
        i += 2

    with tile.TileContext(nc) as tc:
        with (
            tc.tile_pool(name="cons", bufs=1) as cons,
            tc.tile_pool(name="acts", bufs=1) as acts,
            tc.tile_pool(name="work", bufs=2) as work,
            tc.tile_pool(name="ps", bufs=2, space="PSUM") as ps,
            tc.tile_pool(name="ps2", bufs=2, space="PSUM") as ps2,
        ):
            wA2 = cons.tile([PART, 384], F32R, tag="wA2")
            nc.sync.dma_start(wA2[:, :], wA2_d[:, :])
            wB2 = cons.tile([PART, 1], F32R, tag="wB2")
            nc.sync.dma_start(wB2[:, :], wB2_d[:, :])
            bias2 = cons.tile([PART, 4], F32, tag="bias2")
            nc.sync.dma_start(bias2[:, :], bias2_d[:, :])
            wrow = cons.tile([1, 128], F32R, tag="wrow")
            nc.sync.dma_start(wrow[:, :], wrow_d[:, :])
            # gather pairs + small tensors first: the reduce chain that
            # gates phi_h must not queue behind the 3.2MB hT load
            gt = acts.tile([PART, NIDX * 2], F32, tag="gt")
            nc.sync.dma_start(gt[:, :], gt_d[:, :])
            recip = acts.tile([PART, SLOT], F32, tag="recip")
            nc.sync.dma_start(recip[:, :], recip_d[:, :])
            xnl = acts.tile([PART, SLOT * 3], F32, tag="xnl")
            nc.sync.dma_start(xnl[:, :], xnl_d[:, :])
            vnl = acts.tile([PART, SLOT * 3], F32, tag="vnl")
            nc.sync.dma_start(vnl[:, :], vnl_d[:, :])
            hT = acts.tile([PART, NPAD], F32R, tag="hT")
            for poff, pcs0, pcs1 in pairs:
                psl = slice(poff, poff + pcs0 + pcs1)
                nc.sync.dma_start(hT[:, psl], hT_d[:, psl])

            pvrow = acts.tile([1, NPAD], F32, tag="pvrow")
            mrow = acts.tile([1, NPAD], F32R, tag="mrow")

            # ---- segment reduction of gathered pairs (DVE, early) ----
            gt_r = gt[:, :].rearrange("p (s k t) -> p s k t", s=SLOT, k=K, t=2)
            m_i = acts.tile([PART, SLOT], F32, tag="m_i")
            nc.vector.tensor_reduce(m_i[:, :], gt_r[:, :, :, 0],
                                    axis=mybir.AxisListType.X, op=ALU.add)
            mr = acts.tile([PART, SLOT], F32, tag="mr")
            nc.vector.tensor_reduce(mr[:, :], gt_r[:, :, :, 1],
                                    axis=mybir.AxisListType.X, op=ALU.add)
            media = acts.tile([PART, SLOT], F32, tag="media")
            nc.vector.tensor_tensor(media[:, :], mr[:, :], recip[:, :],
                                    op=ALU.mult)
            m_ir = acts.tile([PART, SLOT], F32R, tag="m_ir")
            nc.vector.tensor_copy(m_ir[:, :], m_i[:, :])
            nc.sync.dma_start(mrow[:, :], m_ir[:, :])

            # ---- phi_v over all pairs (dense PE stream) ----
            for off, cs0, cs1 in pairs:
                pw = cs0 + cs1
                halves = [(off, 0, cs0)] + ([(off + cs0, cs0, cs1)] if cs1 else [])
                pA = ps.tile([PART, 1024], F32, tag="mm")
                for ho, po, hcs in halves:
                    r = slice(ho, ho + hcs)
                    o = slice(po, po + hcs)
                    nc.tensor.matmul(pA[:, o], wA2[:, 0:128], hT[:, r],
                                     start=True, stop=True)
                av = work.tile([PART, 1024], F32R, tag="av")
                nc.scalar.activation(av[:, :pw], pA[:, :pw], FT.Tanh,
                                     bias=bias2[:, 0:1])
                for ho, po, hcs in halves:
                    o = slice(po, po + hcs)
                    nc.tensor.matmul(pA[0:1, o], wB2[:, 0:1], av[:, o],
                                     start=True, stop=True)
                nc.vector.tensor_scalar_add(pvrow[:, off:off + pw],
                                            pA[0:1, :pw], bias2[0:1, 3:4])


            # ---- node-major reshuffle + velocity / position update ----
            pvnl = acts.tile([PART, SLOT], F32, tag="pvnl")
            nc.sync.dma_start(pvnl[:, :], pvrow[:, :])
            vn = acts.tile([PART, SLOT * 3], F32, tag="vn")
            xn = acts.tile([PART, SLOT * 3], F32, tag="xn")
            vn_r = vn[:, :].rearrange("p (s t) -> p s t", s=SLOT, t=3)
            xn_r = xn[:, :].rearrange("p (s t) -> p s t", s=SLOT, t=3)
            v_r = vnl[:, :].rearrange("p (s t) -> p s t", s=SLOT, t=3)
            x_r = xnl[:, :].rearrange("p (s t) -> p s t", s=SLOT, t=3)
            nc.vector.tensor_tensor(vn_r, v_r, _bcast_inner(pvnl[:, :], 3),
                                    op=ALU.mult)
            nc.vector.tensor_tensor(vn_r, vn_r, _bcast_inner(media[:, :], 3),
                                    op=ALU.add)
            nc.vector.tensor_tensor(xn_r, x_r, vn_r, op=ALU.add)
            nc.sync.dma_start(
                vnew_d.ap().rearrange("(p s) t -> p (s t)", p=PART), vn[:, :]
            )
            nc.sync.dma_start(
                xnew_d.ap().rearrange("(p s) t -> p (s t)", p=PART), xn[:, :]
            )

            # ---- phi_h over chunk pairs ----
            for off, cs0, cs1 in pairs:
                pw = cs0 + cs1
                halves = [(off, 0, cs0)] + ([(off + cs0, cs0, cs1)] if cs1 else [])
                pC = ps.tile([PART, 1024], F32, tag="mm")
                for ho, po, hcs in halves:
                    r = slice(ho, ho + hcs)
                    o = slice(po, po + hcs)
                    nc.tensor.matmul(pC[:, o], wA2[:, 128:256], hT[:, r],
                                     start=True, stop=False)
                    nc.tensor.matmul(pC[:, o], wrow[0:1, :], mrow[:, r],
                                     start=False, stop=True)
                ah = work.tile([PART, 1024], F32R, tag="ah")
                nc.scalar.activation(ah[:, :pw], pC[:, :pw], FT.Tanh,
                                     bias=bias2[:, 1:2])
                pD = ps2.tile([PART, 1024], F32, tag="mmD")
                for ho, po, hcs in halves:
                    o = slice(po, po + hcs)
                    nc.tensor.matmul(pD[:, o], wA2[:, 256:384], ah[:, o],
                                     start=True, stop=True)
                hnc = work.tile([PART, 1024], F32, tag="hnc")
                nc.vector.tensor_scalar_add(hnc[:, :pw], pD[:, :pw],
                                            bias2[:, 2:3])
                nc.sync.dma_start(hnew_d[:, off:off + pw], hnc[:, :pw])

            if dbg:
                nc.sync.dma_start(mrow_d.ap(), mrow[:, :])
                nc.sync.dma_start(mi_d.ap(), m_i[:, :])

    nc.compile()
    _BUILD_CACHE[key] = nc
    return nc


def _trace_flag():
    return bool(int(os.environ.get("KERNEL_TRACE", "0")))


def kernel(h, x, velocidade, atributos_arestas, rows, cols,
           We1, be1, We2, be2, Wx1, bx1, Wx2, bx2,
           Wh1, bh1, Wh2, bh2, Wv1, bv1, Wv2, bv2):
    LAST_RESULTS.clear()
    h = np.ascontiguousarray(np.asarray(h, dtype=np.float32))
    x = np.ascontiguousarray(np.asarray(x, dtype=np.float32))
    vel = np.ascontiguousarray(np.asarray(velocidade, dtype=np.float32))
    ea = np.asarray(atributos_arestas, dtype=np.float32)
    rows = np.asarray(rows).astype(np.int64)
    cols = np.asarray(cols).astype(np.int64)
    Ws = {k: np.asarray(v, dtype=np.float32) for k, v in dict(
        We1=We1, be1=be1, We2=We2, be2=be2, Wx1=Wx1, bx1=bx1, Wx2=Wx2,
        bx2=bx2, Wh1=Wh1, bh1=bh1, Wh2=Wh2, bh2=bh2, Wv1=Wv1, bv1=bv1,
        Wv2=Wv2, bv2=bv2).items()}
    E = rows.shape[0]
    assert h.shape == (N, FIN) and E >= N

    trace = _trace_flag()
    cores = list(range(NCORES))

    # ---------------- host-side sharding / halo gather ----------------
    r0 = rows[:N]
    c0 = cols[:N]
    diff0 = x[r0] - x[c0]                         # [N, 3]
    dsq0 = diff0 * diff0
    sd0 = diff0.sum(axis=1).astype(np.float32)    # [N]

    # Launch A weight stacks (shared by all cores); matmul weights are
    # pre-rounded to tf32 for the float32r PE fast path
    wA = np.zeros((PART, 512), np.float32)
    wA[:, 0:128] = Ws["We1"][0:128]
    wA[:, 128:256] = Ws["We1"][128:256]
    wA[:, 256:384] = Ws["We2"]
    wA[:, 384:512] = Ws["Wx1"]
    wA = tf32_round(wA)
    wB = np.zeros((PART, 2), np.float32)
    wB[:, 0] = Ws["Wx2"][:, 0]
    wB[:, 1] = 1.0
    wB = tf32_round(wB)
    bias = np.zeros((PART, 4), np.float32)
    bias[:, 0] = Ws["be1"]
    bias[:, 1] = Ws["be2"]
    bias[:, 2] = Ws["bx1"]
    bias[0, 3] = Ws["bx2"][0]
    smallw = np.zeros((IJ + 3, PART), np.float32)
    smallw[0:IJ] = Ws["We1"][257:273]
    smallw[IJ:IJ + 3] = np.tile(Ws["We1"][256:257], (3, 1))
    smallw = tf32_round(smallw)

    def padT(a2d):  # [rows<=SH, d] -> [d, NPAD] transposed + padded
        out = np.zeros((a2d.shape[1], NPAD), np.float32)
        out[:, :a2d.shape[0]] = a2d.T
        return out

    in_maps_a = []
    for c in cores:
        sl = slice(c * SH, (c + 1) * SH)
        edgeT = np.zeros((IJ + 4, NPAD), np.float32)
        edgeT[0:IJ] = tf32_round(padT(ea[:N][sl]))
        edgeT[IJ:IJ + 3] = tf32_round(padT(dsq0[sl]))
        edgeT[IJ + 3] = padT(sd0[sl, None])[0]
        in_maps_a.append({
            "hrT": tf32_round(padT(h[r0[sl]])),
            "hcT": tf32_round(padT(h[c0[sl]])),
            "edgeT": edgeT,
            "wA": wA,
            "wB": wB,
            "bias": bias,
            "smallw": smallw,
        })

    nc_a = _build_launch_a()
    res_a = run_bass_kernel_spmd(nc_a, in_maps_a, core_ids=cores, trace=trace)
    LAST_RESULTS.append(res_a)

    # ---------------- assemble gather table (host reshard) ----------------
    table = np.zeros((N + 1, 2), np.float32)
    for c in cores:
        so = res_a.results[c]["sout"]            # [2, NPAD]
        table[c * SH:(c + 1) * SH, 0] = so[0, :SH]
        table[c * SH:(c + 1) * SH, 1] = so[1, :SH]

    # ---------------- launch B host prep ----------------
    counts = np.bincount(rows, minlength=N).astype(np.int64)
    uniform = E == N * 16 and (counts == 16).all()
    if uniform:
        K = 16
        idx_full = cols.reshape(N, 16)
        pad_mask = None
    else:
        K = max(int(counts.max()), 1)
        K += (-K) % 2
        idx_full = np.full((N, K), N, np.int64)
        offs = np.zeros(N + 1, np.int64)
        np.cumsum(counts, out=offs[1:])
        ar = np.arange(E) - offs[rows]
        idx_full[rows, ar] = cols
    recip_full = np.zeros(N, np.float32)
    nz = counts > 0
    recip_full[nz] = 1.0 / counts[nz]

    # Launch B weights
    wA2 = np.zeros((PART, 384), np.float32)
    wA2[:, 0:128] = Ws["Wv1"]
    wA2[:, 128:256] = Ws["Wh1"][0:128]
    wA2[:, 256:384] = Ws["Wh2"]
    wA2 = tf32_round(wA2)
    wB2 = tf32_round(Ws["Wv2"].reshape(PART, 1))
    bias2 = np.zeros((PART, 4), np.float32)
    bias2[:, 0] = Ws["bv1"]
    bias2[:, 1] = Ws["bh1"]
    bias2[:, 2] = Ws["bh2"]
    bias2[0, 3] = Ws["bv2"][0]
    wrow = tf32_round(Ws["Wh1"][128].reshape(1, 128))

    def node_layout(a, d):  # [<=SH, d] -> [128, SLOT*d]
        out = np.zeros((NPAD, d), np.float32)
        out[:a.shape[0]] = a.reshape(a.shape[0], d)
        return out.reshape(PART, SLOT * d)

    in_maps_b = []
    for c in cores:
        sl = slice(c * SH, (c + 1) * SH)
        idx_sh = np.full((NPAD, K), N, np.int64)
        idx_sh[:SH] = idx_full[sl]
        # host-side value gather (index-driven reshard of launch-A output)
        gt_host = table[idx_sh.reshape(-1)].reshape(PART, SLOT * K * 2)
        in_maps_b.append({
            "hT": tf32_round(padT(h[sl])),
            "xnl": node_layout(x[sl], 3),
            "vnl": node_layout(vel[sl], 3),
            "recip": node_layout(recip_full[sl, None], 1),
            "gt": gt_host,
            "wA2": wA2,
            "wB2": wB2,
            "bias2": bias2,
            "wrow": wrow,
        })

    nc_b = _build_launch_b(K)
    res_b = run_bass_kernel_spmd(nc_b, in_maps_b, core_ids=cores, trace=trace)
    LAST_RESULTS.append(res_b)

    # ---------------- unshard ----------------
    h_new = np.empty((N, FOUT), np.float32)
    x_new = np.empty((N, 3), np.float32)
    v_new = np.empty((N, 3), np.float32)
    for c in cores:
        r = res_b.results[c]
        sl = slice(c * SH, (c + 1) * SH)
        h_new[sl] = r["hnew"][:, :SH].T
        x_new[sl] = r["xnew"][:SH]
        v_new[sl] = r["vnew"][:SH]
    return h_new, x_new, v_new
